# revision 6
# baseline (speedup 1.0000x reference)
"""Multi-head attention (B=2, L=2048, D=1024, H=16, RoPE) on 8 TRN2 NeuronCores.

Sharding: 32 (batch, head) pairs / 8 cores -> core c handles batch c//4 and
heads 4*(c%4) .. 4*(c%4)+3. QKV / out projections are column/row split per
head group; the inter-head-group sum of out-projection partials (plus the
qkv v-bias folded through Wout, and bout) is applied on the host during
unshard (partials are written bf16; the host sums them in fp32).

Per-core dataflow (all matmul operands bf16; PSUM accumulates fp32). The
span is bounded below by the PE column budget (~385k moving columns at
2.4 cols/ns) with the scalar-engine exp stream (~125us) hidden inside it,
so the schedule aims to (a) start the exp stream as early as the input DMA
allows, (b) keep the PE queue dense with fine-grained filler work, and
(c) drain the out-projection early so the tail after the last exp is short:
  - phase A DMAs xT on the SP queue and weights/tables on the gpsimd queue
    (wv/wout last - they are needed late), then projects BOTH pair-0
    m-tiles (k first) dt-outer across all 8 PSUM banks so PE consumption
    tracks the xT/wqk arrival order. RoPE evacuation is chunked per 512
    columns, k/q interleaved, so the first S chunks fire right after the
    first k and q rope chunks land.
  - phase B runs units = (q-chunk, pair) in the order (0,0),(1,0),(2,0),
    (0,1),(1,1),(2,1),(3,0),(3,1), with the PV+normalize of unit i-1
    deferred behind unit i's S/exp stream. Per unit: 32 S^T chunks of
    [128,512] (tile_position packs the two heads onto disjoint PE row
    halves) fill a 2-deep PSUM ring of [128,1536] tiles; one
    raw-immediate-bias EXP covers each ring tile. A single fine-grained
    filler queue (16 v-projection units, then pair-1's q/k projections
    split per-512-column lc, then out-projection units as their q-chunk
    normalizes) is popped one per ring tile plus two after each PV block.
  - normalize: PV accumulators evacuate to SBUF; the denominators (PV row
    64, from the ones column of the v stationaries) go through a DVE
    reciprocal on [1,512] and a 2-hop DRAM broadcast bounce (gpsimd queue);
    the LAST unit instead broadcasts via a tiny ones-stationary matmul so
    no DMA latency sits on the critical tail.
"""
import sys
import numpy as np
import ml_dtypes

try:
    import concourse.bass as bass  # noqa: F401
except ImportError:
    sys.path.insert(0, "/opt/trn_rl_repo")

import concourse.bass as bass
import concourse.mybir as mybir
import concourse.tile as tile
from concourse import bacc
from concourse.bass_utils import run_bass_kernel_spmd

B, L, D = 2, 2048, 1024
H = 16                     # total heads
HPC = 4                    # heads per core
HD = 64                    # head dim
N_CORES = 8
ROPE_BASE = 10000.0

F32 = mybir.dt.float32
BF16 = mybir.dt.bfloat16

LC = 512                   # matmul moving-dim chunk
NLC = L // LC              # 4
NLT = L // 128             # 16 L tiles
NDT = D // 128             # 8 contraction tiles for projections
QK = 2 * HPC * HD          # 512 rows of q+k features
NMT = QK // 128            # 4 m-tiles (0,1 = q heads 0-3; 2,3 = k heads 0-3)
VF = HPC * HD              # 256 v features


def _build_nc():
    nc = bacc.Bacc("TRN2", target_bir_lowering=False, debug=False,
                   num_devices=N_CORES)

    xT_e = nc.declare_dram_parameter("xT", [D, L], BF16, isOutput=False)
    wqk_e = nc.declare_dram_parameter("wqk", [D, QK], BF16, isOutput=False)
    wv_e = nc.declare_dram_parameter("wv", [D, VF], BF16, isOutput=False)
    wout_e = nc.declare_dram_parameter("wout", [VF, D], BF16, isOutput=False)
    cos2_e = nc.declare_dram_parameter("cos2", [128, L], BF16, isOutput=False)
    sin2_e = nc.declare_dram_parameter("sin2", [128, L], BF16, isOutput=False)
    bqk_e = nc.declare_dram_parameter("bqk", [128, NMT], F32, isOutput=False)
    out_e = nc.declare_dram_parameter("out", [L, D], BF16, isOutput=True)

    with tile.TileContext(nc) as tc:
        import contextlib
        with contextlib.ExitStack() as stack:
            persist = stack.enter_context(tc.tile_pool(name="persist", bufs=1))
            dram = stack.enter_context(
                tc.tile_pool(name="dram", bufs=2, space="DRAM"))

            # ---- persistent tiles ------------------------------------------
            qkT = [persist.tile([128, L], BF16, tag=f"qkT{i}", name=f"qkT{i}")
                   for i in range(NMT)]
            v_sb = [persist.tile([128, HPC * (HD + 1)], BF16,
                                 tag=f"v{i}", name=f"v{i}") for i in range(NLT)]
            otT = [persist.tile([128, L], BF16, tag=f"otT{i}", name=f"otT{i}")
                   for i in range(2)]
            wout_sb = [persist.tile([128, D], BF16, tag=f"wout{i}",
                                    name=f"wout{i}") for i in range(2)]
            cos2 = persist.tile([128, L], BF16, tag="cos2")
            sin2 = persist.tile([128, L], BF16, tag="sin2")
            bqk_sb = persist.tile([128, NMT], F32, tag="bqk")
            ones64 = persist.tile([1, HD], BF16, tag="ones64")
            xT_sb = [persist.tile([128, L], BF16, tag=f"xT{i}", name=f"xT{i}")
                     for i in range(NDT)]
            wqk_sb = [persist.tile([128, QK], BF16, tag=f"wqk{i}",
                                   name=f"wqk{i}") for i in range(NDT)]
            wv_sb = [persist.tile([128, VF], BF16, tag=f"wv{i}",
                                  name=f"wv{i}") for i in range(NDT)]

            # ---- phase A ---------------------------------------------------
            with tc.tile_pool(name="qkpsum", bufs=8, space="PSUM") as qkpsum, \
                 tc.tile_pool(name="ptmp", bufs=4) as ptmp:
                # input DMAs: xT alone on the SP queue (first-exp critical
                # path); everything else on the gpsimd queue ordered by need
                # time (wqk tracks xT for the dt-outer projection; cos/sin
                # needed at first rope ~15us; wv at ~22us; wout at ~60us).
                for i in range(NDT):
                    nc.sync.dma_start(out=xT_sb[i], in_=xT_e[i * 128:(i + 1) * 128, :])
                    nc.gpsimd.dma_start(out=wqk_sb[i], in_=wqk_e[i * 128:(i + 1) * 128, :])
                nc.gpsimd.dma_start(out=bqk_sb, in_=bqk_e[:, :])
                nc.gpsimd.dma_start(out=cos2, in_=cos2_e[:, :])
                nc.gpsimd.dma_start(out=sin2, in_=sin2_e[:, :])
                for i in range(NDT):
                    nc.gpsimd.dma_start(out=wv_sb[i], in_=wv_e[i * 128:(i + 1) * 128, :])
                nc.gpsimd.dma_start(out=wout_sb[0], in_=wout_e[0:128, :])
                nc.gpsimd.dma_start(out=wout_sb[1], in_=wout_e[128:256, :])

                # ones column of each v stationary tile (col 64 per head) via
                # memset (replaces 16 slow strided DMAs), plus the [1,64]
                # ones row for the last-unit normalize broadcast matmul.
                for lt in range(NLT):
                    nc.vector.memset(
                        v_sb[lt].rearrange("p (h e) -> p h e", h=HPC)[:, :, HD:HD + 1],
                        1.0)
                nc.vector.memset(ones64, 1.0)

                # Pair-0 projection: BOTH m-tiles (k=mt2, q=mt0) dt-outer
                # across 8 PSUM banks so PE consumption tracks the xT/wqk
                # DMA arrival order; k's accumulations first within each dt.
                pss = {}
                for mt in (2, 0):
                    for lc in range(NLC):
                        pss[(mt, lc)] = qkpsum.tile(
                            [128, LC], F32, tag="qkps",
                            name=f"qkps{mt}_{lc}")

                # PE warmup: junk matmuls ramp the Tensor p-state while the
                # first input DMAs are still in flight. They scribble on the
                # LAST-consumed projection accumulator (q lc3), whose real
                # dt0 matmul (start=True, resets PSUM) runs long after warm.
                warm = ptmp.tile([128, LC], BF16, tag="warm", name="warm")
                nc.vector.memset(warm, 1.0)
                for _ in range(6):
                    nc.tensor.matmul(pss[(0, 3)], warm[:, 0:128], warm,
                                     start=True, stop=True)
                for dt_ in range(NDT):
                    for mt in (2, 0):
                        for lc in range(NLC):
                            nc.tensor.matmul(
                                pss[(mt, lc)],
                                wqk_sb[dt_][:, mt * 128:(mt + 1) * 128],
                                xT_sb[dt_][:, lc * LC:(lc + 1) * LC],
                                start=(dt_ == 0),
                                stop=(dt_ == NDT - 1))

                # Chunked evac + rope, k/q interleaved per lc chunk so the
                # first S chunks need only (k lc0, q lc0).
                t0s = {mt: ptmp.tile([128, L], BF16, tag=f"t0_{mt}",
                                     name=f"t0_{mt}")
                       for mt in (2, 0)}
                for lc in range(NLC):
                    for mt in (2, 0):
                        span = slice(lc * LC, (lc + 1) * LC)
                        t0 = t0s[mt]
                        nc.scalar.activation(
                            out=t0[:, span], in_=pss[(mt, lc)],
                            func=mybir.ActivationFunctionType.Identity,
                            bias=bqk_sb[:, mt:mt + 1], scale=1.0)
                        ta = ptmp.tile([128, LC], BF16, tag="ta",
                                       bufs=3, name=f"ta{mt}_{lc}")
                        nc.vector.tensor_mul(ta, t0[:, span], cos2[:, span])
                        tb = ptmp.tile([128, LC], BF16, tag="tb",
                                       bufs=3, name=f"tb{mt}_{lc}")
                        # rotate_half: out block o0 reads input block i0=o0^32;
                        # sin2 is indexed by the INPUT block (host-prearranged)
                        for blk in range(4):
                            o0 = blk * 32
                            i0 = (blk ^ 1) * 32
                            nc.vector.tensor_mul(
                                tb[o0:o0 + 32, :], t0[i0:i0 + 32, span],
                                sin2[i0:i0 + 32, span])
                        nc.vector.tensor_add(qkT[mt][:, span], ta, tb)

            # ---- phase B: attention, software-pipelined -------------------
            # Units = (qc, hp) with PV deferred ONE UNIT behind S/exp. A
            # single fine filler queue (v units, pair-1 per-lc projections,
            # out-projection units) is popped one per ring tile + two after
            # each PV block.
            CPT = 3                      # chunks per ring tile
            NCH = 2 * NLT                # 32 chunks per (pair, q-chunk)
            with tc.tile_pool(name="e_pool", bufs=24) as e_pool, \
                 tc.tile_pool(name="spsum", bufs=2, space="PSUM") as spsum, \
                 tc.tile_pool(name="opsum", bufs=2, space="PSUM") as opsum, \
                 tc.tile_pool(name="btmp", bufs=2) as btmp:
                tiles = []
                c0 = 0
                while c0 < NCH:
                    tiles.append((c0, min(CPT, NCH - c0)))
                    c0 += CPT
                NT = len(tiles)

                fillq = []

                def pop_fill(n=1):
                    for _ in range(n):
                        if fillq:
                            fillq.pop(0)()

                def emit_s_exp(qc, hp, skip_pops=0):
                    qt = qkT[hp]
                    kt_t = qkT[2 + hp]
                    qs = slice(qc * LC, (qc + 1) * LC)
                    ets = []
                    for t in range(NT):
                        # unit 0's first rings must not queue behind the v
                        # fillers' wv-DMA waits (PE executes in order)
                        if t >= skip_pops:
                            pop_fill()
                        tc0, nch = tiles[t]
                        st = spsum.tile([128, CPT * LC], F32, tag="stps",
                                        name=f"st{hp}_{qc}_{t}")
                        for i in range(nch):
                            c = tc0 + i
                            kt, h = c // 2, c % 2
                            ks = slice(kt * 128, (kt + 1) * 128)
                            rows = slice(h * HD, (h + 1) * HD)
                            nc.tensor.matmul(
                                st[:, i * LC:(i + 1) * LC],
                                kt_t[rows, ks], qt[rows, qs],
                                start=True, stop=True,
                                tile_position=(h * HD, 0))
                        e_t = e_pool.tile([128, CPT * LC], BF16,
                                          tag="e", name=f"e{hp}_{qc}_{t}")
                        ets.append(e_t)
                        eng = nc.scalar
                        eng.add_instruction(mybir.InstActivation(
                            name=nc.get_next_instruction_name(),
                            func=mybir.ActivationFunctionType.Exp,
                            ins=[
                                eng.lower_ap(st[:, 0:nch * LC]),
                                mybir.ImmediateValue(
                                    dtype=mybir.dt.float32, value=0.0),
                                mybir.ImmediateValue(
                                    dtype=mybir.dt.float32,
                                    value=float(HD) ** -0.5),
                                mybir.ImmediateValue(
                                    dtype=mybir.dt.float32, value=0.0),
                            ],
                            outs=[eng.lower_ap(e_t[:, 0:nch * LC])]))
                    return ets

                def emit_pv_norm(qc, hp, ets, last=False):
                    vcs = [slice(h * (HD + 1), (h + 1) * (HD + 1))
                           for h in (2 * hp, 2 * hp + 1)]
                    qs = slice(qc * LC, (qc + 1) * LC)
                    ot_ps = [opsum.tile([128, LC], F32, tag="acc",
                                        name=f"ot{h}_{hp}_{qc}")
                             for h in range(2)]
                    for t in range(NT):
                        tc0, nch = tiles[t]
                        for i in range(nch):
                            c = tc0 + i
                            kt, h = c // 2, c % 2
                            nc.tensor.matmul(
                                ot_ps[h][0:HD + 1, :],
                                v_sb[kt][:, vcs[h]],
                                ets[t][:, i * LC:(i + 1) * LC],
                                start=(kt == 0), stop=(kt == NLT - 1))
                    ot_sb = [btmp.tile([HD + 1, LC], F32, tag="otsb",
                                       bufs=4, name=f"osb{h}_{hp}_{qc}")
                             for h in range(2)]
                    for h in range(2):
                        nc.vector.tensor_copy(out=ot_sb[h],
                                              in_=ot_ps[h][0:HD + 1, :])
                    if not last:
                        # 2-hop DRAM broadcast bounce on the gpsimd queue:
                        # reciprocal on [1,512] per head, write both to one
                        # DRAM row, broadcast-read [64,512] per head.
                        rrec = btmp.tile([1, 2 * LC], F32, tag="rrec",
                                         name=f"rrec{hp}_{qc}")
                        for h in range(2):
                            nc.vector.reciprocal(
                                out=rrec[0:1, h * LC:(h + 1) * LC],
                                in_=ot_sb[h][HD:HD + 1, :])
                        d2 = dram.tile([1, 2 * LC], F32, tag="d2",
                                       name=f"d2_{hp}_{qc}")
                        nc.gpsimd.dma_start(out=d2, in_=rrec)
                        for h in range(2):
                            bc_sb = btmp.tile([HD, LC], F32, tag="bcsb",
                                              name=f"bc{h}_{hp}_{qc}")
                            bcast_src = bass.AP(
                                tensor=d2.tensor, offset=d2.offset + h * LC,
                                ap=[[0, HD], [1, LC]])
                            nc.gpsimd.dma_start(out=bc_sb, in_=bcast_src)
                            nc.vector.tensor_mul(
                                otT[hp][h * HD:(h + 1) * HD, qs],
                                ot_sb[h][0:HD, :], bc_sb)
                    else:
                        # last unit: no DMA on the critical tail. bf16
                        # reciprocal broadcast via a ones-stationary matmul.
                        for h in range(2):
                            rr = btmp.tile([1, LC], BF16, tag="rrb",
                                           name=f"rrb{h}_{hp}_{qc}")
                            with nc.allow_low_precision(
                                    reason="bf16 1/denom on the last unit "
                                           "only; ~0.4% on 1/8 of tokens"):
                                nc.vector.reciprocal(
                                    out=rr, in_=ot_sb[h][HD:HD + 1, :])
                            bc_ps = opsum.tile([128, LC], F32, tag="acc",
                                               name=f"bcps{h}_{hp}_{qc}")
                            nc.tensor.matmul(bc_ps[0:HD, :],
                                             ones64, rr,
                                             start=True, stop=True)
                            nc.vector.tensor_mul(
                                otT[hp][h * HD:(h + 1) * HD, qs],
                                ot_sb[h][0:HD, :], bc_ps[0:HD, :])

                def project_qk_lc_b(mt, lc):
                    # pair-1 qk projection, one 512-column chunk: fine
                    # filler so the exp stream never starves behind it.
                    ps = opsum.tile([128, LC], F32, tag="acc",
                                    name=f"bqkps{mt}_{lc}")
                    for dt_ in range(NDT):
                        nc.tensor.matmul(
                            ps,
                            wqk_sb[dt_][:, mt * 128:(mt + 1) * 128],
                            xT_sb[dt_][:, lc * LC:(lc + 1) * LC],
                            start=(dt_ == 0), stop=(dt_ == NDT - 1))
                    span = slice(lc * LC, (lc + 1) * LC)
                    t0 = btmp.tile([128, LC], BF16, tag="bt0", bufs=3,
                                   name=f"bt0_{mt}_{lc}")
                    nc.vector.tensor_scalar(
                        t0, ps, bqk_sb[:, mt:mt + 1], None,
                        mybir.AluOpType.add)
                    ta = btmp.tile([128, LC], BF16, tag="bta", bufs=3,
                                   name=f"bta_{mt}_{lc}")
                    nc.vector.tensor_mul(ta, t0, cos2[:, span])
                    tb = btmp.tile([128, LC], BF16, tag="btb", bufs=3,
                                   name=f"btb_{mt}_{lc}")
                    for blk in range(4):
                        o0 = blk * 32
                        i0 = (blk ^ 1) * 32
                        nc.vector.tensor_mul(
                            tb[o0:o0 + 32, :], t0[i0:i0 + 32, :],
                            sin2[i0:i0 + 32, span])
                    nc.vector.tensor_add(qkT[mt][:, span], ta, tb)

                def project_v_b(lt):
                    # v projection: acc-bank rotation, DVE evac (the ACT
                    # engine is saturated with exps here). The v bias is
                    # folded through Wout on the host, so no bias matmul.
                    ps = opsum.tile([128, LC], F32, tag="acc",
                                    name=f"vps{lt}")
                    for dt_ in range(NDT):
                        nc.tensor.matmul(
                            ps[:, 0:VF],
                            xT_sb[dt_][:, lt * 128:(lt + 1) * 128],
                            wv_sb[dt_],
                            start=(dt_ == 0), stop=(dt_ == NDT - 1))
                    nc.vector.tensor_copy(
                        out=v_sb[lt].rearrange("p (h e) -> p h e",
                                               h=HPC)[:, :, 0:HD],
                        in_=ps[:, 0:VF].rearrange("p (h e) -> p h e", h=HPC))

                def make_c_unit(lt, nch):
                    # out-projection unit, woven into later units' PE slack
                    def unit():
                        yps = opsum.tile([128, LC], F32, tag="acc",
                                         name=f"yps{lt}_{nch}")
                        for ft in range(2):
                            nc.tensor.matmul(
                                yps,
                                otT[ft][:, lt * 128:(lt + 1) * 128],
                                wout_sb[ft][:, nch * LC:(nch + 1) * LC],
                                start=(ft == 0), stop=(ft == 1))
                        y_sb = btmp.tile([128, LC], BF16,
                                         tag="ysb", bufs=4,
                                         name=f"ysb{lt}_{nch}")
                        nc.vector.tensor_copy(out=y_sb, in_=yps)
                        nc.sync.dma_start(
                            out=out_e[lt * 128:(lt + 1) * 128,
                                      nch * LC:(nch + 1) * LC],
                            in_=y_sb)
                    return unit

                units = [(0, 0), (1, 0), (2, 0), (0, 1),
                         (1, 1), (2, 1), (3, 0), (3, 1)]
                # filler order: v units (needed by PV of unit 0, which runs
                # during unit 1), then pair-1 k (mt3) per-lc, then pair-1 q
                # (mt1) per-lc - all complete before unit 3 = (0,1) needs
                # them. c-units are appended as their normalize lands.
                fillq.extend(
                    [(lambda lt=lt: project_v_b(lt)) for lt in range(NLT)])
                fillq.extend(
                    [(lambda lc=lc: project_qk_lc_b(3, lc)) for lc in range(NLC)])
                fillq.extend(
                    [(lambda lc=lc: project_qk_lc_b(1, lc)) for lc in range(NLC)])

                prev = None
                for i, (qc, hp) in enumerate(units):
                    ets = emit_s_exp(qc, hp, skip_pops=3 if i == 0 else 0)
                    if prev is not None:
                        emit_pv_norm(prev[0], prev[1], prev[2])
                        if prev[1] == 1:
                            for j in range(4):
                                for nch in range(2):
                                    fillq.append(
                                        make_c_unit(prev[0] * 4 + j, nch))
                        # late fillers AFTER the PV block: its acc-bank
                        # allocation never queues behind a filler evac
                        pop_fill(2)
                    prev = (qc, hp, ets)
                emit_pv_norm(prev[0], prev[1], prev[2], last=True)
                for j in range(4):
                    for nch in range(2):
                        fillq.append(make_c_unit(prev[0] * 4 + j, nch))
                pop_fill(len(fillq))

    nc.compile()
    return nc


def _rope_tables():
    inv_freq = 1.0 / (ROPE_BASE ** (np.arange(0, HD, 2, dtype=np.float32) / HD))
    t = np.arange(L, dtype=np.float32)
    freqs = np.einsum("i,j->ij", t, inv_freq)            # [L, 32]
    emb = np.concatenate((freqs, freqs), axis=-1)        # [L, 64]
    cosT = np.cos(emb).T.astype(np.float32)              # [64, L]
    sinT = np.sin(emb).T.astype(np.float32)              # [64, L]
    # sin table is indexed by the INPUT partition of the rotate_half term:
    # out[0:32] reads in[32:64] -> table rows 32:64 hold -sin;
    # out[32:64] reads in[0:32] -> table rows 0:32 hold +sin.
    cos2 = np.concatenate([cosT, cosT], axis=0)          # [128, L]
    sin_signed = np.concatenate([sinT[:32], -sinT[32:]], axis=0)  # [64, L]
    sin2 = np.concatenate([sin_signed, sin_signed], axis=0)       # [128, L]
    return np.ascontiguousarray(cos2), np.ascontiguousarray(sin2)


_NC = None
TRACE = False          # test harness sets True to collect exec_time_ns
LAST_RESULT = None


def kernel(x, Wqkv, bqkv, Wout, bout):
    global _NC, LAST_RESULT
    if _NC is None:
        _NC = _build_nc()

    x = np.asarray(x, dtype=np.float32)
    Wqkv = np.asarray(Wqkv, dtype=np.float32)
    bqkv = np.asarray(bqkv, dtype=np.float32)
    Wout = np.asarray(Wout, dtype=np.float32)
    bout = np.asarray(bout, dtype=np.float32)

    cos2, sin2 = _rope_tables()

    in_maps = []
    for c in range(N_CORES):
        b = c // 4
        heads = [4 * (c % 4) + i for i in range(HPC)]
        xT = np.ascontiguousarray(x[b].T)                            # [D, L]
        q_cols = [Wqkv[:, h * HD:(h + 1) * HD] for h in heads]
        k_cols = [Wqkv[:, D + h * HD:D + (h + 1) * HD] for h in heads]
        v_cols = [Wqkv[:, 2 * D + h * HD:2 * D + (h + 1) * HD] for h in heads]
        wqk = np.ascontiguousarray(np.concatenate(q_cols + k_cols, axis=1))
        wv = np.ascontiguousarray(np.concatenate(v_cols, axis=1))
        bq = np.concatenate([bqkv[h * HD:(h + 1) * HD] for h in heads])
        bk = np.concatenate([bqkv[D + h * HD:D + (h + 1) * HD] for h in heads])
        wout = np.ascontiguousarray(
            np.concatenate([Wout[h * HD:(h + 1) * HD, :] for h in heads],
                           axis=0))
        in_maps.append({
            "xT": xT.astype(ml_dtypes.bfloat16),
            "wqk": wqk.astype(ml_dtypes.bfloat16),
            "wv": wv.astype(ml_dtypes.bfloat16),
            "wout": wout.astype(ml_dtypes.bfloat16),
            "cos2": cos2.astype(ml_dtypes.bfloat16),
            "sin2": sin2.astype(ml_dtypes.bfloat16),
            "bqk": np.ascontiguousarray(
                np.concatenate([bq, bk]).reshape(NMT, 128).T),
        })

    res = run_bass_kernel_spmd(_NC, in_maps, core_ids=list(range(N_CORES)),
                               trace=TRACE)
    LAST_RESULT = res

    out = np.zeros((B, L, D), dtype=np.float32)
    for c in range(N_CORES):
        out[c // 4] += res.results[c]["out"].astype(np.float32)
    # v-bias folded through Wout (attention rows sum to 1), plus bout
    out += (bqkv[2 * D:] @ Wout)[None, None, :] + bout[None, None, :]
    return out


# revision 14
# speedup vs baseline: 1.0675x; 1.0675x over previous
"""Multi-head attention (B=2, L=2048, D=1024, H=16, RoPE) on 8 TRN2 NeuronCores.

Sharding: 32 (batch, head) pairs / 8 cores -> core c handles batch c//4 and
heads 4*(c%4) .. 4*(c%4)+3. QKV / out projections are column/row split per
head group; the inter-head-group sum of out-projection partials (plus the
qkv v-bias folded through Wout, and bout) is applied on the host during
unshard (partials are written bf16; the host sums them in fp32).

Per-core dataflow (all matmul operands bf16; PSUM accumulates fp32). The
span is bounded below by the PE column budget (~385k moving columns at
2.4 cols/ns) with the scalar-engine exp stream (~125us) hidden inside it,
so the schedule aims to (a) start the exp stream as early as the input DMA
allows, (b) keep the PE queue dense with fine-grained filler work, and
(c) drain the out-projection early so the tail after the last exp is short:
  - phase A DMAs xT on the SP queue and weights/tables on the gpsimd queue
    (wv/wout last - they are needed late), then projects BOTH pair-0
    m-tiles (k first) dt-outer across all 8 PSUM banks so PE consumption
    tracks the xT/wqk arrival order. RoPE evacuation is chunked per 512
    columns, k/q interleaved, so the first S chunks fire right after the
    first k and q rope chunks land.
  - phase B runs units = (q-chunk, pair) in the order (0,0),(1,0),(2,0),
    (0,1),(1,1),(2,1),(3,0),(3,1), with the PV+normalize of unit i-1
    deferred behind unit i's S/exp stream. Per unit: 32 S^T chunks of
    [128,512] (tile_position packs the two heads onto disjoint PE row
    halves) fill a 2-deep PSUM ring of [128,1536] tiles; one
    raw-immediate-bias EXP covers each ring tile. A single fine-grained
    filler queue (16 v-projection units, then pair-1's q/k projections
    split per-512-column lc, then out-projection units as their q-chunk
    normalizes) is popped one per ring tile plus two after each PV block.
  - normalize: PV accumulators evacuate to SBUF; the denominators (PV row
    64, from the ones column of the v stationaries) go through a DVE
    reciprocal on [1,512] and a 2-hop DRAM broadcast bounce (gpsimd queue);
    the LAST unit instead broadcasts via a tiny ones-stationary matmul so
    no DMA latency sits on the critical tail.
"""
import sys
import numpy as np
import ml_dtypes

try:
    import concourse.bass as bass  # noqa: F401
except ImportError:
    sys.path.insert(0, "/opt/trn_rl_repo")

import concourse.bass as bass
import concourse.mybir as mybir
import concourse.tile as tile
from concourse import bacc
from concourse.bass_utils import run_bass_kernel_spmd

B, L, D = 2, 2048, 1024
H = 16                     # total heads
HPC = 4                    # heads per core
HD = 64                    # head dim
N_CORES = 8
ROPE_BASE = 10000.0

F32 = mybir.dt.float32
BF16 = mybir.dt.bfloat16

LC = 512                   # matmul moving-dim chunk
NLC = L // LC              # 4
NLT = L // 128             # 16 L tiles
NDT = D // 128             # 8 contraction tiles for projections
QK = 2 * HPC * HD          # 512 rows of q+k features
NMT = QK // 128            # 4 m-tiles (0,1 = q heads 0-3; 2,3 = k heads 0-3)
VF = HPC * HD              # 256 v features


def _build_nc():
    nc = bacc.Bacc("TRN2", target_bir_lowering=False, debug=False,
                   num_devices=N_CORES)

    xT_e = nc.declare_dram_parameter("xT", [D, L], BF16, isOutput=False)
    wqk_e = nc.declare_dram_parameter("wqk", [D, QK], BF16, isOutput=False)
    wv_e = nc.declare_dram_parameter("wv", [D, VF], BF16, isOutput=False)
    wout_e = nc.declare_dram_parameter("wout", [VF, D], BF16, isOutput=False)
    # rope tables ship at their unique sizes (rows repeat in 32/64 blocks)
    # and are replicated on-chip - saves 640KB of the first-exp-critical DMA
    cosS_e = nc.declare_dram_parameter("cosS", [32, L], BF16, isOutput=False)
    sinS_e = nc.declare_dram_parameter("sinS", [64, L], BF16, isOutput=False)
    bqk_e = nc.declare_dram_parameter("bqk", [128, NMT], F32, isOutput=False)
    out_e = nc.declare_dram_parameter("out", [L, D], BF16, isOutput=True)

    with tile.TileContext(nc) as tc:
        import contextlib
        with contextlib.ExitStack() as stack:
            persist = stack.enter_context(tc.tile_pool(name="persist", bufs=1))
            dram = stack.enter_context(
                tc.tile_pool(name="dram", bufs=2, space="DRAM"))

            # ---- persistent tiles ------------------------------------------
            qkT = [persist.tile([128, L], BF16, tag=f"qkT{i}", name=f"qkT{i}")
                   for i in range(NMT)]
            v_sb = [persist.tile([128, HPC * (HD + 1)], BF16,
                                 tag=f"v{i}", name=f"v{i}") for i in range(NLT)]
            otT = [persist.tile([128, L], BF16, tag=f"otT{i}", name=f"otT{i}")
                   for i in range(2)]
            wout_sb = [persist.tile([128, D], BF16, tag=f"wout{i}",
                                    name=f"wout{i}") for i in range(2)]
            cos2 = persist.tile([128, L], BF16, tag="cos2")
            sin2 = persist.tile([128, L], BF16, tag="sin2")
            bqk_sb = persist.tile([128, NMT], F32, tag="bqk")
            xT_sb = [persist.tile([128, L], BF16, tag=f"xT{i}", name=f"xT{i}")
                     for i in range(NDT)]
            wqk_sb = [persist.tile([128, QK], BF16, tag=f"wqk{i}",
                                   name=f"wqk{i}") for i in range(NDT)]
            wv_sb = [persist.tile([128, VF], BF16, tag=f"wv{i}",
                                  name=f"wv{i}") for i in range(NDT)]

            # ---- phase A ---------------------------------------------------
            with tc.tile_pool(name="qkpsum", bufs=8, space="PSUM") as qkpsum, \
                 tc.tile_pool(name="ptmp", bufs=4) as ptmp:
                # input DMAs. The DMA fabric is ~300GB/s aggregate across all
                # queues, so what matters is the BYTE ORDER: first-exp needs
                # xT+wqk+cos/sin+bqk (5.4MB); wv/wout (1MB) are deferred via
                # the scalar queue below, AFTER the first evacuations (which
                # postdate the last xT tile), so they never compete.
                for i in range(NDT):
                    nc.sync.dma_start(out=xT_sb[i], in_=xT_e[i * 128:(i + 1) * 128, :])
                    nc.gpsimd.dma_start(out=wqk_sb[i], in_=wqk_e[i * 128:(i + 1) * 128, :])
                nc.gpsimd.dma_start(out=bqk_sb, in_=bqk_e[:, :])
                nc.gpsimd.dma_start(out=cos2[0:32, :], in_=cosS_e[:, :])
                nc.gpsimd.dma_start(out=sin2[0:64, :], in_=sinS_e[:, :])
                # on-chip replication of the repeating rope-table rows
                nc.gpsimd.dma_start(out=cos2[32:64, :], in_=cos2[0:32, :])
                nc.gpsimd.dma_start(out=cos2[64:128, :], in_=cos2[0:64, :])
                nc.gpsimd.dma_start(out=sin2[64:128, :], in_=sin2[0:64, :])

                # ones column of each v stationary tile (col 64 per head) via
                # memset (replaces 16 slow strided DMAs).
                for lt in range(NLT):
                    nc.vector.memset(
                        v_sb[lt].rearrange("p (h e) -> p h e", h=HPC)[:, :, HD:HD + 1],
                        1.0)

                # Pair-0 projection: BOTH m-tiles (k=mt2, q=mt0) dt-outer
                # across 8 PSUM banks so PE consumption tracks the xT/wqk
                # DMA arrival order; k's accumulations first within each dt.
                pss = {}
                for mt in (2, 0):
                    for lc in range(NLC):
                        pss[(mt, lc)] = qkpsum.tile(
                            [128, LC], F32, tag="qkps",
                            name=f"qkps{mt}_{lc}")

                # PE warmup: junk matmuls ramp the Tensor p-state while the
                # first input DMAs are still in flight. They scribble on the
                # LAST-consumed projection accumulator (q lc3), whose real
                # dt0 matmul (start=True, resets PSUM) runs long after warm.
                warm = ptmp.tile([128, LC], BF16, tag="warm", name="warm")
                nc.vector.memset(warm, 1.0)
                for _ in range(6):
                    nc.tensor.matmul(pss[(0, 3)], warm[:, 0:128], warm,
                                     start=True, stop=True)
                for dt_ in range(NDT):
                    for mt in (2, 0):
                        for lc in range(NLC):
                            nc.tensor.matmul(
                                pss[(mt, lc)],
                                wqk_sb[dt_][:, mt * 128:(mt + 1) * 128],
                                xT_sb[dt_][:, lc * LC:(lc + 1) * LC],
                                start=(dt_ == 0),
                                stop=(dt_ == NDT - 1))

                # Chunked evac + rope, k/q interleaved per lc chunk so the
                # first S chunks need only (k lc0, q lc0).
                t0s = {mt: ptmp.tile([128, L], BF16, tag=f"t0_{mt}",
                                     name=f"t0_{mt}")
                       for mt in (2, 0)}
                for lc in range(NLC):
                    if lc == 1:
                        # deferred wv/wout DMA triggers: the scalar queue
                        # reaches these only after the lc0 evacuations,
                        # which postdate the last xT tile - so these bytes
                        # never compete with the first-exp-critical set.
                        for i in range(NDT):
                            nc.scalar.dma_start(
                                out=wv_sb[i], in_=wv_e[i * 128:(i + 1) * 128, :])
                        nc.scalar.dma_start(out=wout_sb[0], in_=wout_e[0:128, :])
                        nc.scalar.dma_start(out=wout_sb[1], in_=wout_e[128:256, :])
                    for mt in (2, 0):
                        span = slice(lc * LC, (lc + 1) * LC)
                        t0 = t0s[mt]
                        nc.scalar.activation(
                            out=t0[:, span], in_=pss[(mt, lc)],
                            func=mybir.ActivationFunctionType.Identity,
                            bias=bqk_sb[:, mt:mt + 1], scale=1.0)
                        ta = ptmp.tile([128, LC], BF16, tag="ta",
                                       bufs=3, name=f"ta{mt}_{lc}")
                        nc.vector.tensor_mul(ta, t0[:, span], cos2[:, span])
                        tb = ptmp.tile([128, LC], BF16, tag="tb",
                                       bufs=3, name=f"tb{mt}_{lc}")
                        # rotate_half: out block o0 reads input block i0=o0^32;
                        # sin2 is indexed by the INPUT block (host-prearranged)
                        for blk in range(4):
                            o0 = blk * 32
                            i0 = (blk ^ 1) * 32
                            nc.vector.tensor_mul(
                                tb[o0:o0 + 32, :], t0[i0:i0 + 32, span],
                                sin2[i0:i0 + 32, span])
                        nc.vector.tensor_add(qkT[mt][:, span], ta, tb)

            # ---- phase B: attention, software-pipelined -------------------
            # Units = (qc, hp) with PV deferred ONE UNIT behind S/exp. A
            # single fine filler queue (v units, pair-1 per-lc projections,
            # out-projection units) is popped one per ring tile + two after
            # each PV block.
            CPT = 3                      # chunks per ring tile
            NCH = 2 * NLT                # 32 chunks per (pair, q-chunk)
            with tc.tile_pool(name="e_pool", bufs=24) as e_pool, \
                 tc.tile_pool(name="spsum", bufs=2, space="PSUM") as spsum, \
                 tc.tile_pool(name="opsum", bufs=2, space="PSUM") as opsum, \
                 tc.tile_pool(name="btmp", bufs=2) as btmp:
                tiles = []
                c0 = 0
                while c0 < NCH:
                    tiles.append((c0, min(CPT, NCH - c0)))
                    c0 += CPT
                NT = len(tiles)

                fillq = []

                def pop_fill(n=1):
                    for _ in range(n):
                        if fillq:
                            fillq.pop(0)()

                def emit_s_exp(qc, hp, skip_pops=0):
                    qt = qkT[hp]
                    kt_t = qkT[2 + hp]
                    qs = slice(qc * LC, (qc + 1) * LC)
                    ets = []
                    for t in range(NT):
                        # unit 0's first rings must not queue behind the v
                        # fillers' wv-DMA waits (PE executes in order)
                        if t >= skip_pops:
                            pop_fill()
                        tc0, nch = tiles[t]
                        st = spsum.tile([128, CPT * LC], F32, tag="stps",
                                        name=f"st{hp}_{qc}_{t}")
                        for i in range(nch):
                            c = tc0 + i
                            kt, h = c // 2, c % 2
                            ks = slice(kt * 128, (kt + 1) * 128)
                            rows = slice(h * HD, (h + 1) * HD)
                            nc.tensor.matmul(
                                st[:, i * LC:(i + 1) * LC],
                                kt_t[rows, ks], qt[rows, qs],
                                start=True, stop=True,
                                tile_position=(h * HD, 0))
                        e_t = e_pool.tile([128, CPT * LC], BF16,
                                          tag="e", name=f"e{hp}_{qc}_{t}")
                        ets.append(e_t)
                        eng = nc.scalar
                        eng.add_instruction(mybir.InstActivation(
                            name=nc.get_next_instruction_name(),
                            func=mybir.ActivationFunctionType.Exp,
                            ins=[
                                eng.lower_ap(st[:, 0:nch * LC]),
                                mybir.ImmediateValue(
                                    dtype=mybir.dt.float32, value=0.0),
                                mybir.ImmediateValue(
                                    dtype=mybir.dt.float32,
                                    value=float(HD) ** -0.5),
                                mybir.ImmediateValue(
                                    dtype=mybir.dt.float32, value=0.0),
                            ],
                            outs=[eng.lower_ap(e_t[:, 0:nch * LC])]))
                    return ets

                def emit_pv_norm(qc, hp, ets):
                    vcs = [slice(h * (HD + 1), (h + 1) * (HD + 1))
                           for h in (2 * hp, 2 * hp + 1)]
                    qs = slice(qc * LC, (qc + 1) * LC)
                    ot_ps = [opsum.tile([128, LC], F32, tag="acc",
                                        name=f"ot{h}_{hp}_{qc}")
                             for h in range(2)]
                    for t in range(NT):
                        tc0, nch = tiles[t]
                        for i in range(nch):
                            c = tc0 + i
                            kt, h = c // 2, c % 2
                            nc.tensor.matmul(
                                ot_ps[h][0:HD + 1, :],
                                v_sb[kt][:, vcs[h]],
                                ets[t][:, i * LC:(i + 1) * LC],
                                start=(kt == 0), stop=(kt == NLT - 1))
                    ot_sb = [btmp.tile([HD + 1, LC], F32, tag="otsb",
                                       bufs=4, name=f"osb{h}_{hp}_{qc}")
                             for h in range(2)]
                    for h in range(2):
                        nc.vector.tensor_copy(out=ot_sb[h],
                                              in_=ot_ps[h][0:HD + 1, :])
                    # denominator path: the DVE reciprocal only parallelizes
                    # across partitions, so bounce [1,1024] through DRAM into
                    # [128,8] (52ns reciprocal) and broadcast back. All hops
                    # ride the gpsimd queue, which is idle in phase B.
                    d1 = dram.tile([1, 2 * LC], F32, tag="d1",
                                   name=f"d1_{hp}_{qc}")
                    for h in range(2):
                        nc.gpsimd.dma_start(
                            out=d1[0:1, h * LC:(h + 1) * LC],
                            in_=ot_sb[h][HD:HD + 1, :])
                    rsq = btmp.tile([128, 2 * LC // 128], F32, tag="rsq",
                                    name=f"rsq{hp}_{qc}")
                    nc.gpsimd.dma_start(
                        out=rsq,
                        in_=d1.rearrange("o (p f) -> (o p) f", p=128))
                    rrec = btmp.tile([128, 2 * LC // 128], F32,
                                     tag="rrec", name=f"rrec{hp}_{qc}")
                    nc.vector.reciprocal(out=rrec, in_=rsq)
                    d2 = dram.tile([1, 2 * LC], F32, tag="d2",
                                   name=f"d2_{hp}_{qc}")
                    nc.gpsimd.dma_start(
                        out=d2.rearrange("o (p f) -> (o p) f", p=128),
                        in_=rrec)
                    for h in range(2):
                        bc_sb = btmp.tile([HD, LC], F32, tag="bcsb",
                                          name=f"bc{h}_{hp}_{qc}")
                        bcast_src = bass.AP(
                            tensor=d2.tensor, offset=d2.offset + h * LC,
                            ap=[[0, HD], [1, LC]])
                        nc.gpsimd.dma_start(out=bc_sb, in_=bcast_src)
                        nc.vector.tensor_mul(
                            otT[hp][h * HD:(h + 1) * HD, qs],
                            ot_sb[h][0:HD, :], bc_sb)

                def project_qk_lc_b(mt, lc):
                    # pair-1 qk projection, one 512-column chunk: fine
                    # filler so the exp stream never starves behind it.
                    ps = opsum.tile([128, LC], F32, tag="acc",
                                    name=f"bqkps{mt}_{lc}")
                    for dt_ in range(NDT):
                        nc.tensor.matmul(
                            ps,
                            wqk_sb[dt_][:, mt * 128:(mt + 1) * 128],
                            xT_sb[dt_][:, lc * LC:(lc + 1) * LC],
                            start=(dt_ == 0), stop=(dt_ == NDT - 1))
                    span = slice(lc * LC, (lc + 1) * LC)
                    t0 = btmp.tile([128, LC], BF16, tag="bt0", bufs=3,
                                   name=f"bt0_{mt}_{lc}")
                    nc.vector.tensor_scalar(
                        t0, ps, bqk_sb[:, mt:mt + 1], None,
                        mybir.AluOpType.add)
                    ta = btmp.tile([128, LC], BF16, tag="bta", bufs=3,
                                   name=f"bta_{mt}_{lc}")
                    nc.vector.tensor_mul(ta, t0, cos2[:, span])
                    tb = btmp.tile([128, LC], BF16, tag="btb", bufs=3,
                                   name=f"btb_{mt}_{lc}")
                    for blk in range(4):
                        o0 = blk * 32
                        i0 = (blk ^ 1) * 32
                        nc.vector.tensor_mul(
                            tb[o0:o0 + 32, :], t0[i0:i0 + 32, :],
                            sin2[i0:i0 + 32, span])
                    nc.vector.tensor_add(qkT[mt][:, span], ta, tb)

                def project_v_b(lt):
                    # v projection: acc-bank rotation, DVE evac (the ACT
                    # engine is saturated with exps here). The v bias is
                    # folded through Wout on the host, so no bias matmul.
                    ps = opsum.tile([128, LC], F32, tag="acc",
                                    name=f"vps{lt}")
                    for dt_ in range(NDT):
                        nc.tensor.matmul(
                            ps[:, 0:VF],
                            xT_sb[dt_][:, lt * 128:(lt + 1) * 128],
                            wv_sb[dt_],
                            start=(dt_ == 0), stop=(dt_ == NDT - 1))
                    nc.vector.tensor_copy(
                        out=v_sb[lt].rearrange("p (h e) -> p h e",
                                               h=HPC)[:, :, 0:HD],
                        in_=ps[:, 0:VF].rearrange("p (h e) -> p h e", h=HPC))

                def make_c_unit(lt, nch):
                    # out-projection unit, woven into later units' PE slack
                    def unit():
                        yps = opsum.tile([128, LC], F32, tag="acc",
                                         name=f"yps{lt}_{nch}")
                        for ft in range(2):
                            nc.tensor.matmul(
                                yps,
                                otT[ft][:, lt * 128:(lt + 1) * 128],
                                wout_sb[ft][:, nch * LC:(nch + 1) * LC],
                                start=(ft == 0), stop=(ft == 1))
                        y_sb = btmp.tile([128, LC], BF16,
                                         tag="ysb", bufs=4,
                                         name=f"ysb{lt}_{nch}")
                        nc.vector.tensor_copy(out=y_sb, in_=yps)
                        nc.sync.dma_start(
                            out=out_e[lt * 128:(lt + 1) * 128,
                                      nch * LC:(nch + 1) * LC],
                            in_=y_sb)
                    return unit

                units = [(0, 0), (1, 0), (2, 0), (0, 1),
                         (1, 1), (2, 1), (3, 0), (3, 1)]
                # filler order: v units (needed by PV of unit 0, which runs
                # during unit 1), then pair-1 k (mt3) per-lc, then pair-1 q
                # (mt1) per-lc - all complete before unit 3 = (0,1) needs
                # them. c-units are appended as their normalize lands.
                fillq.extend(
                    [(lambda lt=lt: project_v_b(lt)) for lt in range(NLT)])
                fillq.extend(
                    [(lambda lc=lc: project_qk_lc_b(3, lc)) for lc in range(NLC)])
                fillq.extend(
                    [(lambda lc=lc: project_qk_lc_b(1, lc)) for lc in range(NLC)])

                prev = None
                for i, (qc, hp) in enumerate(units):
                    ets = emit_s_exp(qc, hp, skip_pops=3 if i == 0 else 0)
                    if prev is not None:
                        emit_pv_norm(prev[0], prev[1], prev[2])
                        if prev[1] == 1:
                            for j in range(4):
                                for nch in range(2):
                                    fillq.append(
                                        make_c_unit(prev[0] * 4 + j, nch))
                        # late fillers AFTER the PV block: its acc-bank
                        # allocation never queues behind a filler evac
                        pop_fill(2)
                    prev = (qc, hp, ets)
                emit_pv_norm(prev[0], prev[1], prev[2])
                for j in range(4):
                    for nch in range(2):
                        fillq.append(make_c_unit(prev[0] * 4 + j, nch))
                pop_fill(len(fillq))

    nc.compile()
    return nc


def _rope_tables():
    inv_freq = 1.0 / (ROPE_BASE ** (np.arange(0, HD, 2, dtype=np.float32) / HD))
    t = np.arange(L, dtype=np.float32)
    freqs = np.einsum("i,j->ij", t, inv_freq)            # [L, 32]
    emb = np.concatenate((freqs, freqs), axis=-1)        # [L, 64]
    cosT = np.cos(emb).T.astype(np.float32)              # [64, L]
    sinT = np.sin(emb).T.astype(np.float32)              # [64, L]
    # sin table is indexed by the INPUT partition of the rotate_half term:
    # out[0:32] reads in[32:64] -> table rows 32:64 hold -sin;
    # out[32:64] reads in[0:32] -> table rows 0:32 hold +sin.
    cos2 = np.concatenate([cosT, cosT], axis=0)          # [128, L]
    sin_signed = np.concatenate([sinT[:32], -sinT[32:]], axis=0)  # [64, L]
    sin2 = np.concatenate([sin_signed, sin_signed], axis=0)       # [128, L]
    return np.ascontiguousarray(cos2), np.ascontiguousarray(sin2)


_NC = None
TRACE = False          # test harness sets True to collect exec_time_ns
LAST_RESULT = None


def kernel(x, Wqkv, bqkv, Wout, bout):
    global _NC, LAST_RESULT
    if _NC is None:
        _NC = _build_nc()

    x = np.asarray(x, dtype=np.float32)
    Wqkv = np.asarray(Wqkv, dtype=np.float32)
    bqkv = np.asarray(bqkv, dtype=np.float32)
    Wout = np.asarray(Wout, dtype=np.float32)
    bout = np.asarray(bout, dtype=np.float32)

    cos2, sin2 = _rope_tables()

    in_maps = []
    for c in range(N_CORES):
        b = c // 4
        heads = [4 * (c % 4) + i for i in range(HPC)]
        xT = np.ascontiguousarray(x[b].T)                            # [D, L]
        q_cols = [Wqkv[:, h * HD:(h + 1) * HD] for h in heads]
        k_cols = [Wqkv[:, D + h * HD:D + (h + 1) * HD] for h in heads]
        v_cols = [Wqkv[:, 2 * D + h * HD:2 * D + (h + 1) * HD] for h in heads]
        wqk = np.ascontiguousarray(np.concatenate(q_cols + k_cols, axis=1))
        wv = np.ascontiguousarray(np.concatenate(v_cols, axis=1))
        bq = np.concatenate([bqkv[h * HD:(h + 1) * HD] for h in heads])
        bk = np.concatenate([bqkv[D + h * HD:D + (h + 1) * HD] for h in heads])
        wout = np.ascontiguousarray(
            np.concatenate([Wout[h * HD:(h + 1) * HD, :] for h in heads],
                           axis=0))
        in_maps.append({
            "xT": xT.astype(ml_dtypes.bfloat16),
            "wqk": wqk.astype(ml_dtypes.bfloat16),
            "wv": wv.astype(ml_dtypes.bfloat16),
            "wout": wout.astype(ml_dtypes.bfloat16),
            "cosS": cos2[0:32].astype(ml_dtypes.bfloat16),
            "sinS": sin2[0:64].astype(ml_dtypes.bfloat16),
            "bqk": np.ascontiguousarray(
                np.concatenate([bq, bk]).reshape(NMT, 128).T),
        })

    res = run_bass_kernel_spmd(_NC, in_maps, core_ids=list(range(N_CORES)),
                               trace=TRACE)
    LAST_RESULT = res

    out = np.zeros((B, L, D), dtype=np.float32)
    for c in range(N_CORES):
        out[c // 4] += res.results[c]["out"].astype(np.float32)
    # v-bias folded through Wout (attention rows sum to 1), plus bout
    out += (bqkv[2 * D:] @ Wout)[None, None, :] + bout[None, None, :]
    return out
